# revision 32
# baseline (speedup 1.0000x reference)
"""Trainium2 Bass kernel for fused ragged attention pooling (v2).

Problem: single-query multihead attention pooling over segmented nodes.
N=131072 nodes, D=512, B=512 graphs, H=8 heads; graded instance regular:
graph g owns nodes [256*g, 256*(g+1)).

Math (exact, weights folded on host):
    scores[n,h] = x[n,:] @ A[:,h]
    p           = segment softmax(scores)
    S[gh, i]    = sum_{n in g} p[n,h] * x[n,i]
    out         = (S @ WvT per head) @ Wout^T + bias

v2 design (vs v1 which PE-transposed x on chip):
  - Host uploads TWO fp16 copies of x: natural-layout tiles (pooling moving
    operand) and pre-transposed tiles (scores moving operand). This deletes
    the on-chip PE transpose stream AND the PSUM->SBUF staging copies that
    dominated Vector/Scalar time, at the cost of 2x HBM read traffic.
    Both layouts give >=4KB contiguous runs per partition -> near-peak DMA.
  - Softmax: exp on Scalar straight from the scores PSUM (with accum_out
    denominators); normalization is folded into the p-transpose matmul by
    streaming diag(1/den) as the moving operand (regular matmul).
  - Pooling: block-diagonal p16 stationary (16 graphs / 128 cols), exactly
    one x stream through the PE.
"""

import numpy as np

N, D, B, H = 131072, 512, 512, 8
DH = D // H            # 64
CORES = 8
GPC = B // CORES       # graphs per core = 64
NPG = N // B           # nodes per graph = 256
GROUP = 16             # graphs per pooling group
NPGRP = GROUP * NPG    # nodes per group = 4096

_CACHE = {}

CONF = {
    "xtdt": "e3m4",    # dtype of the transposed (scores) copy: f16 | e3m4
}


def _xt_np_dtype(conf):
    if conf["xtdt"] == "e3m4":
        import ml_dtypes

        return ml_dtypes.float8_e3m4
    return np.float16


def _in_maps(x, A4, WvT4, Wout8, conf=None):
    """Per-core input dicts. x must already be fp16 [N, D]."""
    conf = dict(CONF, **(conf or {}))
    x = np.asarray(x, np.float16)
    npc = GPC * NPG
    grps = GPC // GROUP
    ident = np.eye(128, dtype=np.float16)
    xtdt = _xt_np_dtype(conf)
    maps = []
    for c in range(CORES):
        xc = x[c * npc : (c + 1) * npc]
        xg = np.ascontiguousarray(
            xc.reshape(grps, NPGRP // 128, 128, D).transpose(0, 2, 1, 3)
        )
        # pair-major transposed copy: [grp, pair, i_part, c, n_in_pair]
        xt = np.ascontiguousarray(
            xc.reshape(grps, 8, 512, 4, 128).transpose(0, 1, 4, 3, 2)
        ).astype(xtdt)
        maps.append(
            {
                "xg": xg,
                "xt": xt,
                "a4": A4.astype(np.float16),
                "wvt4": WvT4.astype(np.float16),
                "wout8": Wout8,
                "identr": ident,
            }
        )
    return maps


def _build(n_graphs=GPC, repeat=1, variant="full", **overrides):
    """Build + compile the per-core Bass program."""
    conf = dict(CONF, **overrides)
    from contextlib import ExitStack, nullcontext

    import concourse.bacc as bacc
    import concourse.tile as tile
    from concourse import mybir

    F32 = mybir.dt.float32
    F32R = mybir.dt.float32r
    F16 = mybir.dt.float16
    U32 = mybir.dt.uint32
    EXP = mybir.ActivationFunctionType.Exp
    XTD = mybir.dt.float8e3 if conf["xtdt"] == "e3m4" else F16

    assert n_graphs % GROUP == 0
    n_groups = n_graphs // GROUP
    n_pairs = GROUP // 2           # pairs per group = 8
    CHG = 2 * GROUP                # node-chunks per group = 32

    nc = bacc.Bacc("TRN2", target_bir_lowering=False, debug=False)

    xg_d = nc.dram_tensor("xg", [n_groups, 128, CHG, D], F16, kind="ExternalInput")
    xt_d = nc.dram_tensor(
        "xt", [n_groups, n_pairs, 128, 4, 512], XTD, kind="ExternalInput"
    )
    a4_d = nc.dram_tensor("a4", [128, 4, H], F16, kind="ExternalInput")
    wv_d = nc.dram_tensor("wvt4", [128, 4, H, DH], F16, kind="ExternalInput")
    wo_d = nc.dram_tensor("wout8", [DH, H, D], F32R, kind="ExternalInput")
    idr_d = nc.dram_tensor("identr", [128, 128], F16, kind="ExternalInput")
    out_d = nc.dram_tensor("out", [n_graphs, D], F32, kind="ExternalOutput")

    with tile.TileContext(nc) as tc, ExitStack() as ctx:
        const = ctx.enter_context(tc.tile_pool(name="const", bufs=1))
        xt_pool = ctx.enter_context(tc.tile_pool(name="xt", bufs=3))
        xg_pool = ctx.enter_context(tc.tile_pool(name="xgp", bufs=2))
        p16_pool = ctx.enter_context(tc.tile_pool(name="p16", bufs=1))
        small = ctx.enter_context(tc.tile_pool(name="small", bufs=6))
        s2sb_pool = ctx.enter_context(tc.tile_pool(name="s2sb", bufs=2))
        stall_pool = ctx.enter_context(tc.tile_pool(name="stall", bufs=1))
        tail_sb = ctx.enter_context(tc.tile_pool(name="tailsb", bufs=1))
        # PSUM: 8 banks: sc 2 + pnat 2 + s2 2 + tail 1 + pool4 1 = 8
        sc_pool = ctx.enter_context(tc.tile_pool(name="scps", bufs=2, space="PSUM"))
        pnat_pool = ctx.enter_context(tc.tile_pool(name="pnat", bufs=2, space="PSUM"))
        s2_pool = ctx.enter_context(tc.tile_pool(name="s2ps", bufs=2, space="PSUM"))
        tail_ps = ctx.enter_context(tc.tile_pool(name="tailps", bufs=1, space="PSUM"))
        pool4_ps = ctx.enter_context(tc.tile_pool(name="pool4ps", bufs=1, space="PSUM"))

        A4 = const.tile([128, 4, H], F16)
        nc.sync.dma_start(A4[:], a4_d[:])
        WvT4 = const.tile([128, 4, H, DH], F16)
        nc.sync.dma_start(WvT4[:], wv_d[:])
        Wout8 = const.tile([DH, H, D], F32R)
        nc.sync.dma_start(Wout8[:], wo_d[:])
        identr = const.tile([128, 128], F16)
        nc.sync.dma_start(identr[:], idr_d[:])
        ones = const.tile([128, 1], F16)
        nc.vector.memset(ones[:], 1.0)
        nbias = const.tile([128, 1], F32)
        nc.vector.memset(nbias[:], -2.0)

        # persistent block-diagonal p matrices; stripe slots are rewritten
        # every group, the zeros persist from this one-time memset.
        P16 = [
            p16_pool.tile([128, CHG, 128], F16, tag=f"p16_{i}", name=f"p16_{i}")
            for i in range(min(2, n_groups))
        ]
        for t in P16:
            nc.vector.memset(t[:].bitcast(U32), 0)
        # [i, c, h, grp, gl]: pool4's moving slice [:, c, h, :, :] is contiguous
        STall = stall_pool.tile([128, 4, H, n_groups, GROUP], F16)
        POOL4 = pool4_ps.tile([DH, H, n_graphs], F32, name="pool4")

        loop_cm = tc.For_i(0, repeat, 1) if repeat > 1 else nullcontext()
        with loop_cm:
            for grp in range(n_groups):
                # quarter-group loads (2 pairs each) for fine-grained overlap
                xt = xt_pool.tile([128, n_pairs, 4, 512], XTD, tag="xt")
                for q in range(4):
                    nc.sync.dma_start(
                        xt[:, 2 * q : 2 * q + 2, :, :],
                        xt_d[grp][2 * q : 2 * q + 2].rearrange(
                            "a p c d -> p a c d", p=128
                        ),
                    )
                xg = xg_pool.tile([128, CHG, D], F16, tag="xg")
                for q in range(4):
                    nc.scalar.dma_start(
                        xg[:, 8 * q : 8 * q + 8, :], xg_d[grp][:, 8 * q : 8 * q + 8, :]
                    )
                if variant == "dma":
                    continue
                p16 = P16[grp % len(P16)]
                s2ps = s2_pool.tile([128, D], F32, tag="s2")
                # one-pair software-pipeline skew: pair p+1's scores matmuls
                # are emitted BEFORE pair p's transposes/pooling, so the PE
                # has independent work to run while pair p's softmax chain
                # (Scalar exp -> DVE recip/mul) completes.
                pend = {}
                for pstep in range(n_pairs + 1):
                    if pstep < n_pairs:
                        p = pstep
                        scps = sc_pool.tile([H, 2, NPG], F32, tag="sc")
                        for c in range(4):
                            nc.tensor.matmul(
                                scps[:],
                                A4[:, c, :],
                                xt[:, p, c, :],
                                start=(c == 0),
                                stop=(c == 3),
                            )
                        # exp + per-graph denominators, then p = e/den on DVE
                        eT = small.tile([H, 2, NPG], F32, tag="eT")
                        den = small.tile([H, 2, 1], F32, tag="den")
                        rden = small.tile([H, 2, 1], F32, tag="rden")
                        pT = small.tile([H, 2, NPG], F16, tag="pT")
                        for s in range(2):
                            nc.scalar.activation(
                                eT[:, s, :], scps[:, s, :], EXP,
                                accum_out=den[:, s, :],
                            )
                        nc.vector.reciprocal(rden[:], den[:])
                        for s in range(2):
                            nc.vector.tensor_scalar_mul(
                                pT[:, s, :], eT[:, s, :], rden[:, s, :]
                            )
                        pend[p] = pT
                    if pstep >= 1:
                        p = pstep - 1
                        pT = pend.pop(p)
                        # p natural: 4 transpose-path ops [8,128] -> [128,8]
                        pnat = pnat_pool.tile([128, 2, 2, H], F16, tag="pnat")
                        for k in range(4):
                            s, kk = k // 2, k % 2
                            nc.tensor.matmul(
                                pnat[:, s, kk, :],
                                pT[:, s, 128 * kk : 128 * (kk + 1)],
                                identr[0:H, 0:H],
                                is_transpose=True,
                            )
                        # scatter into block-diag p16: chunk 4p+2s+kk, col h*16+gl
                        from concourse.ap import AP as _AP

                        gl0 = 2 * p
                        dst = _AP(
                            p16.tensor,
                            p16.offset + (4 * p) * 128 + gl0,
                            [list(p16.ap[0])] + [[2 * 128 + 1, 2], [128, 2], [GROUP, H]],
                        )
                        nc.vector.tensor_copy(dst, pnat[:])
                        # pooling: one x stream, block-diag stationary
                        for k in range(4):
                            ch = 4 * p + k
                            nc.tensor.matmul(
                                s2ps[:],
                                p16[:, ch, :],
                                xg[:, ch, :],
                                start=(p == 0 and k == 0),
                                stop=(p == n_pairs - 1 and k == 3),
                            )
                # group tail: evacuate S2, transpose, project (spread per group)
                s2sb = s2sb_pool.tile([128, D], F16, tag="s2sb")
                nc.vector.tensor_copy(s2sb[:], s2ps[:])
                stps = tail_ps.tile([128, 4, H, GROUP], F16, tag="tail")
                for c in range(4):
                    nc.tensor.matmul(
                        stps[:, c, :, :],
                        s2sb[:, 128 * c : 128 * (c + 1)],
                        identr[:],
                        is_transpose=True,
                    )
                nc.scalar.copy(STall[:, :, :, grp, :], stps[:])
                for h in range(H):
                    for c in range(4):
                        nc.tensor.matmul(
                            POOL4[:, h, grp * GROUP : (grp + 1) * GROUP],
                            WvT4[:, c, h, :],
                            STall[:, c, h, grp, :],
                            start=(c == 0),
                            stop=(c == 3),
                        )

            if variant == "dma":
                finz = tail_sb.tile([n_graphs, D], F32, tag="finsb")
                nc.vector.memset(finz[:], 0.0)
                nc.sync.dma_start(out_d[:], finz[:])
            else:
                pool4 = POOL4
                pool4sb = tail_sb.tile([DH, H, n_graphs], F32R, tag="p4sb")
                nc.vector.tensor_copy(pool4sb[:], pool4[:])
                finps = tail_ps.tile([n_graphs, D], F32, tag="tail")
                for h in range(H):
                    nc.tensor.matmul(
                        finps[:],
                        pool4sb[:, h, :],
                        Wout8[:, h, :],
                        start=(h == 0),
                        stop=(h == H - 1),
                    )
                finsb = tail_sb.tile([n_graphs, D], F32, tag="finsb")
                nc.vector.tensor_copy(finsb[:], finps[:])
                nc.sync.dma_start(out_d[:], finsb[:])

    nc.compile()
    _strip_debug(nc)
    return nc


def _strip_debug(nc):
    """Remove source-path debug info so the neuron compile-cache key is
    independent of where this file lives."""
    for fn in nc.m.functions:
        for alloc in fn.allocations:
            try:
                for ml in alloc.memorylocations or []:
                    if getattr(ml, "ant_debug", None) is not None:
                        ml.ant_debug = None
            except Exception:
                pass
        for b in fn.blocks:
            for inst in b.instructions:
                try:
                    if inst.debug is not None:
                        inst.debug = None
                    if inst.bass_addl_debug is not None:
                        inst.bass_addl_debug = None
                except Exception:
                    pass


def _host_prep(query, W_in, b_in, W_out, b_out):
    """Fold the tiny weights into the layouts the device kernel wants."""
    scale = 1.0 / np.sqrt(DH)
    q = ((query @ W_in[:D].T + b_in[:D]) * scale).reshape(H, DH)
    Wk = W_in[D : 2 * D]
    A = (Wk.reshape(H, DH, D) * q[:, :, None]).sum(1).T.astype(np.float32)
    A4 = np.ascontiguousarray(A.reshape(4, 128, H).transpose(1, 0, 2))
    WvT = W_in[2 * D :].T.astype(np.float32)  # [i, j]
    WvT4 = np.ascontiguousarray(WvT.reshape(4, 128, H, DH).transpose(1, 0, 2, 3))
    WoutT = W_out.T.astype(np.float32)  # [j, d]
    Wout8 = np.ascontiguousarray(WoutT.reshape(H, DH, D).transpose(1, 0, 2))
    bias = (W_out @ b_in[2 * D :] + b_out).astype(np.float32)  # [D]
    return A4, WvT4, Wout8, bias


def _numpy_fallback(x, batch, num_graphs, query, W_in, b_in, W_out, b_out):
    """Exact reference math in numpy (handles arbitrary sorted segments)."""
    nb = int(num_graphs)
    scale = 1.0 / np.sqrt(DH)
    q = ((query @ W_in[:D].T + b_in[:D]) * scale).reshape(H, DH)
    k = (x @ W_in[D : 2 * D].T + b_in[D : 2 * D]).reshape(-1, H, DH)
    v = (x @ W_in[2 * D :].T + b_in[2 * D :]).reshape(-1, H, DH)
    scores = np.einsum("nhd,hd->nh", k, q)
    smax = np.full((nb, H), -np.inf, np.float32)
    np.maximum.at(smax, batch, scores)
    e = np.exp(scores - smax[batch])
    denom = np.zeros((nb, H), np.float32)
    np.add.at(denom, batch, e)
    p = e / denom[batch]
    pooled = np.zeros((nb, H, DH), np.float32)
    np.add.at(pooled, batch, p[:, :, None] * v)
    return (pooled.reshape(nb, D) @ W_out.T + b_out).astype(np.float32)


def kernel(**inputs):
    x = np.asarray(inputs["x"], dtype=np.float32)
    batch = np.asarray(inputs["batch"]).astype(np.int64)
    num_graphs = int(np.asarray(inputs["num_graphs"]))
    query = np.asarray(inputs["query"], dtype=np.float32)
    W_in = np.asarray(inputs["W_in"], dtype=np.float32)
    b_in = np.asarray(inputs["b_in"], dtype=np.float32)
    W_out = np.asarray(inputs["W_out"], dtype=np.float32)
    b_out = np.asarray(inputs["b_out"], dtype=np.float32)

    regular = (
        x.shape == (N, D)
        and num_graphs == B
        and batch.shape == (N,)
        and np.array_equal(batch, np.repeat(np.arange(B, dtype=np.int64), NPG))
    )
    if not regular:
        return _numpy_fallback(
            x, batch, num_graphs, query, W_in, b_in, W_out, b_out
        )

    from concourse.bass_utils import run_bass_kernel_spmd

    A4, WvT4, Wout8, bias = _host_prep(query, W_in, b_in, W_out, b_out)

    if "prog" not in _CACHE:
        _CACHE["prog"] = _build(GPC)
    nc = _CACHE["prog"]

    in_maps = _in_maps(x.astype(np.float16), A4, WvT4, Wout8)
    res = run_bass_kernel_spmd(nc, in_maps, list(range(CORES)))
    out = np.concatenate([res.results[c]["out"] for c in range(CORES)], axis=0)
    return (out + bias[None, :]).astype(np.float32)


# revision 33
# speedup vs baseline: 1.0284x; 1.0284x over previous
"""Trainium2 Bass kernel for fused ragged attention pooling (v2).

Problem: single-query multihead attention pooling over segmented nodes.
N=131072 nodes, D=512, B=512 graphs, H=8 heads; graded instance regular:
graph g owns nodes [256*g, 256*(g+1)).

Math (exact, weights folded on host):
    scores[n,h] = x[n,:] @ A[:,h]
    p           = segment softmax(scores)
    S[gh, i]    = sum_{n in g} p[n,h] * x[n,i]
    out         = (S @ WvT per head) @ Wout^T + bias

v2 design (vs v1 which PE-transposed x on chip):
  - Host uploads TWO fp16 copies of x: natural-layout tiles (pooling moving
    operand) and pre-transposed tiles (scores moving operand). This deletes
    the on-chip PE transpose stream AND the PSUM->SBUF staging copies that
    dominated Vector/Scalar time, at the cost of 2x HBM read traffic.
    Both layouts give >=4KB contiguous runs per partition -> near-peak DMA.
  - Softmax: exp on Scalar straight from the scores PSUM (with accum_out
    denominators); normalization is folded into the p-transpose matmul by
    streaming diag(1/den) as the moving operand (regular matmul).
  - Pooling: block-diagonal p16 stationary (16 graphs / 128 cols), exactly
    one x stream through the PE.
"""

import numpy as np

N, D, B, H = 131072, 512, 512, 8
DH = D // H            # 64
CORES = 8
GPC = B // CORES       # graphs per core = 64
NPG = N // B           # nodes per graph = 256
GROUP = 16             # graphs per pooling group
NPGRP = GROUP * NPG    # nodes per group = 4096

_CACHE = {}

CONF = {
    "xtdt": "e3m4",    # dtype of the transposed (scores) copy: f16 | e3m4
}


def _xt_np_dtype(conf):
    if conf["xtdt"] == "e3m4":
        import ml_dtypes

        return ml_dtypes.float8_e3m4
    return np.float16


def _in_maps(x, A4, WvT4, Wout8, conf=None):
    """Per-core input dicts. x must already be fp16 [N, D]."""
    conf = dict(CONF, **(conf or {}))
    x = np.asarray(x, np.float16)
    npc = GPC * NPG
    grps = GPC // GROUP
    ident = np.eye(128, dtype=np.float16)
    xtdt = _xt_np_dtype(conf)
    maps = []
    for c in range(CORES):
        xc = x[c * npc : (c + 1) * npc]
        xg = np.ascontiguousarray(
            xc.reshape(grps, NPGRP // 128, 128, D).transpose(0, 2, 1, 3)
        )
        # pair-major transposed copy: [grp, pair, i_part, c, n_in_pair]
        xt = np.ascontiguousarray(
            xc.reshape(grps, 8, 512, 4, 128).transpose(0, 1, 4, 3, 2)
        ).astype(xtdt)
        maps.append(
            {
                "xg": xg,
                "xt": xt,
                "a4": A4.astype(np.float16),
                "wvt4": WvT4.astype(np.float16),
                "wout8": Wout8,
                "identr": ident,
            }
        )
    return maps


def _build(n_graphs=GPC, repeat=1, variant="full", **overrides):
    """Build + compile the per-core Bass program."""
    conf = dict(CONF, **overrides)
    from contextlib import ExitStack, nullcontext

    import concourse.bacc as bacc
    import concourse.tile as tile
    from concourse import mybir

    F32 = mybir.dt.float32
    F32R = mybir.dt.float32r
    F16 = mybir.dt.float16
    U32 = mybir.dt.uint32
    EXP = mybir.ActivationFunctionType.Exp
    XTD = mybir.dt.float8e3 if conf["xtdt"] == "e3m4" else F16

    assert n_graphs % GROUP == 0
    n_groups = n_graphs // GROUP
    n_pairs = GROUP // 2           # pairs per group = 8
    CHG = 2 * GROUP                # node-chunks per group = 32

    nc = bacc.Bacc("TRN2", target_bir_lowering=False, debug=False)

    xg_d = nc.dram_tensor("xg", [n_groups, 128, CHG, D], F16, kind="ExternalInput")
    xt_d = nc.dram_tensor(
        "xt", [n_groups, n_pairs, 128, 4, 512], XTD, kind="ExternalInput"
    )
    a4_d = nc.dram_tensor("a4", [128, 4, H], F16, kind="ExternalInput")
    wv_d = nc.dram_tensor("wvt4", [128, 4, H, DH], F16, kind="ExternalInput")
    wo_d = nc.dram_tensor("wout8", [DH, H, D], F32R, kind="ExternalInput")
    idr_d = nc.dram_tensor("identr", [128, 128], F16, kind="ExternalInput")
    out_d = nc.dram_tensor("out", [n_graphs, D], F32, kind="ExternalOutput")

    with tile.TileContext(nc) as tc, ExitStack() as ctx:
        const = ctx.enter_context(tc.tile_pool(name="const", bufs=1))
        xt_pool = ctx.enter_context(tc.tile_pool(name="xt", bufs=3))
        xg_pool = ctx.enter_context(tc.tile_pool(name="xgp", bufs=2))
        p16_pool = ctx.enter_context(tc.tile_pool(name="p16", bufs=1))
        small = ctx.enter_context(tc.tile_pool(name="small", bufs=6))
        s2sb_pool = ctx.enter_context(tc.tile_pool(name="s2sb", bufs=2))
        stall_pool = ctx.enter_context(tc.tile_pool(name="stall", bufs=1))
        tail_sb = ctx.enter_context(tc.tile_pool(name="tailsb", bufs=1))
        # PSUM: 8 banks: sc 2 + pnat 2 + s2 2 + tail 1 + pool4 1 = 8
        sc_pool = ctx.enter_context(tc.tile_pool(name="scps", bufs=2, space="PSUM"))
        pnat_pool = ctx.enter_context(tc.tile_pool(name="pnat", bufs=2, space="PSUM"))
        s2_pool = ctx.enter_context(tc.tile_pool(name="s2ps", bufs=2, space="PSUM"))
        tail_ps = ctx.enter_context(tc.tile_pool(name="tailps", bufs=1, space="PSUM"))
        pool4_ps = ctx.enter_context(tc.tile_pool(name="pool4ps", bufs=1, space="PSUM"))

        A4 = const.tile([128, 4, H], F16)
        nc.sync.dma_start(A4[:], a4_d[:])
        WvT4 = const.tile([128, 4, H, DH], F16)
        nc.sync.dma_start(WvT4[:], wv_d[:])
        Wout8 = const.tile([DH, H, D], F32R)
        nc.sync.dma_start(Wout8[:], wo_d[:])
        identr = const.tile([128, 128], F16)
        nc.sync.dma_start(identr[:], idr_d[:])
        ones = const.tile([128, 1], F16)
        nc.vector.memset(ones[:], 1.0)
        nbias = const.tile([128, 1], F32)
        nc.vector.memset(nbias[:], -2.0)

        # persistent block-diagonal p matrices; stripe slots are rewritten
        # every group, the zeros persist from this one-time memset.
        P16 = [
            p16_pool.tile([128, CHG, 128], F16, tag=f"p16_{i}", name=f"p16_{i}")
            for i in range(min(2, n_groups))
        ]
        for t in P16:
            nc.vector.memset(t[:].bitcast(U32), 0)
        # [i, c, h, grp, gl]: pool4's moving slice [:, c, h, :, :] is contiguous
        STall = stall_pool.tile([128, 4, H, n_groups, GROUP], F16)
        POOL4 = pool4_ps.tile([DH, H, n_graphs], F32, name="pool4")

        loop_cm = tc.For_i(0, repeat, 1) if repeat > 1 else nullcontext()
        with loop_cm:
            for grp in range(n_groups):
                # quarter-group loads (2 pairs each) for fine-grained overlap
                xt = xt_pool.tile([128, n_pairs, 4, 512], XTD, tag="xt")
                for q in range(4):
                    nc.sync.dma_start(
                        xt[:, 2 * q : 2 * q + 2, :, :],
                        xt_d[grp][2 * q : 2 * q + 2].rearrange(
                            "a p c d -> p a c d", p=128
                        ),
                    )
                xg = xg_pool.tile([128, CHG, D], F16, tag="xg")
                for q in range(4):
                    nc.scalar.dma_start(
                        xg[:, 8 * q : 8 * q + 8, :], xg_d[grp][:, 8 * q : 8 * q + 8, :]
                    )
                if variant == "dma":
                    continue
                p16 = P16[grp % len(P16)]
                s2ps = s2_pool.tile([128, D], F32, tag="s2")
                # one-pair software-pipeline skew: pair p+1's scores matmuls
                # are emitted BEFORE pair p's transposes/pooling, so the PE
                # has independent work to run while pair p's softmax chain
                # (Scalar exp -> DVE recip/mul) completes.
                pend = {}
                for pstep in range(n_pairs + 1):
                    if pstep < n_pairs:
                        p = pstep
                        scps = sc_pool.tile([H, 2, NPG], F32, tag="sc")
                        for c in range(4):
                            nc.tensor.matmul(
                                scps[:],
                                A4[:, c, :],
                                xt[:, p, c, :],
                                start=(c == 0),
                                stop=(c == 3),
                            )
                        # exp + per-graph denominators, then p = e/den on DVE
                        eT = small.tile([H, 2, NPG], F32, tag="eT")
                        den = small.tile([H, 2, 1], F32, tag="den")
                        rden = small.tile([H, 2, 1], F32, tag="rden")
                        pT = small.tile([H, 2, NPG], F16, tag="pT")
                        for s in range(2):
                            nc.scalar.activation(
                                eT[:, s, :], scps[:, s, :], EXP,
                                accum_out=den[:, s, :],
                            )
                        nc.vector.reciprocal(rden[:], den[:])
                        for s in range(2):
                            nc.vector.tensor_scalar_mul(
                                pT[:, s, :], eT[:, s, :], rden[:, s, :]
                            )
                        pend[p] = pT
                    if pstep >= 1:
                        p = pstep - 1
                        pT = pend.pop(p)
                        # p natural: 4 transpose-path ops [8,128] -> [128,8]
                        pnat = pnat_pool.tile([128, 2, 2, H], F32, tag="pnat")
                        for k in range(4):
                            s, kk = k // 2, k % 2
                            nc.tensor.matmul(
                                pnat[:, s, kk, :],
                                pT[:, s, 128 * kk : 128 * (kk + 1)],
                                identr[0:H, 0:H],
                            )
                        # scatter into block-diag p16: chunk 4p+2s+kk, col h*16+gl
                        from concourse.ap import AP as _AP

                        gl0 = 2 * p
                        dst = _AP(
                            p16.tensor,
                            p16.offset + (4 * p) * 128 + gl0,
                            [list(p16.ap[0])] + [[2 * 128 + 1, 2], [128, 2], [GROUP, H]],
                        )
                        nc.vector.tensor_copy(dst, pnat[:])
                        # pooling: one x stream, block-diag stationary
                        for k in range(4):
                            ch = 4 * p + k
                            nc.tensor.matmul(
                                s2ps[:],
                                p16[:, ch, :],
                                xg[:, ch, :],
                                start=(p == 0 and k == 0),
                                stop=(p == n_pairs - 1 and k == 3),
                            )
                # group tail: evacuate S2, transpose, project (spread per group)
                s2sb = s2sb_pool.tile([128, D], F16, tag="s2sb")
                nc.vector.tensor_copy(s2sb[:], s2ps[:])
                stps = tail_ps.tile([128, 4, H, GROUP], F16, tag="tail")
                for c in range(4):
                    nc.tensor.matmul(
                        stps[:, c, :, :],
                        s2sb[:, 128 * c : 128 * (c + 1)],
                        identr[:],
                        is_transpose=True,
                    )
                nc.scalar.copy(STall[:, :, :, grp, :], stps[:])
                for h in range(H):
                    for c in range(4):
                        nc.tensor.matmul(
                            POOL4[:, h, grp * GROUP : (grp + 1) * GROUP],
                            WvT4[:, c, h, :],
                            STall[:, c, h, grp, :],
                            start=(c == 0),
                            stop=(c == 3),
                        )

            if variant == "dma":
                finz = tail_sb.tile([n_graphs, D], F32, tag="finsb")
                nc.vector.memset(finz[:], 0.0)
                nc.sync.dma_start(out_d[:], finz[:])
            else:
                pool4 = POOL4
                pool4sb = tail_sb.tile([DH, H, n_graphs], F32R, tag="p4sb")
                nc.vector.tensor_copy(pool4sb[:], pool4[:])
                finps = tail_ps.tile([n_graphs, D], F32, tag="tail")
                for h in range(H):
                    nc.tensor.matmul(
                        finps[:],
                        pool4sb[:, h, :],
                        Wout8[:, h, :],
                        start=(h == 0),
                        stop=(h == H - 1),
                    )
                finsb = tail_sb.tile([n_graphs, D], F32, tag="finsb")
                nc.vector.tensor_copy(finsb[:], finps[:])
                nc.sync.dma_start(out_d[:], finsb[:])

    nc.compile()
    _strip_debug(nc)
    return nc


def _strip_debug(nc):
    """Remove source-path debug info so the neuron compile-cache key is
    independent of where this file lives."""
    for fn in nc.m.functions:
        for alloc in fn.allocations:
            try:
                for ml in alloc.memorylocations or []:
                    if getattr(ml, "ant_debug", None) is not None:
                        ml.ant_debug = None
            except Exception:
                pass
        for b in fn.blocks:
            for inst in b.instructions:
                try:
                    if inst.debug is not None:
                        inst.debug = None
                    if inst.bass_addl_debug is not None:
                        inst.bass_addl_debug = None
                except Exception:
                    pass


def _host_prep(query, W_in, b_in, W_out, b_out):
    """Fold the tiny weights into the layouts the device kernel wants."""
    scale = 1.0 / np.sqrt(DH)
    q = ((query @ W_in[:D].T + b_in[:D]) * scale).reshape(H, DH)
    Wk = W_in[D : 2 * D]
    A = (Wk.reshape(H, DH, D) * q[:, :, None]).sum(1).T.astype(np.float32)
    A4 = np.ascontiguousarray(A.reshape(4, 128, H).transpose(1, 0, 2))
    WvT = W_in[2 * D :].T.astype(np.float32)  # [i, j]
    WvT4 = np.ascontiguousarray(WvT.reshape(4, 128, H, DH).transpose(1, 0, 2, 3))
    WoutT = W_out.T.astype(np.float32)  # [j, d]
    Wout8 = np.ascontiguousarray(WoutT.reshape(H, DH, D).transpose(1, 0, 2))
    bias = (W_out @ b_in[2 * D :] + b_out).astype(np.float32)  # [D]
    return A4, WvT4, Wout8, bias


def _numpy_fallback(x, batch, num_graphs, query, W_in, b_in, W_out, b_out):
    """Exact reference math in numpy (handles arbitrary sorted segments)."""
    nb = int(num_graphs)
    scale = 1.0 / np.sqrt(DH)
    q = ((query @ W_in[:D].T + b_in[:D]) * scale).reshape(H, DH)
    k = (x @ W_in[D : 2 * D].T + b_in[D : 2 * D]).reshape(-1, H, DH)
    v = (x @ W_in[2 * D :].T + b_in[2 * D :]).reshape(-1, H, DH)
    scores = np.einsum("nhd,hd->nh", k, q)
    smax = np.full((nb, H), -np.inf, np.float32)
    np.maximum.at(smax, batch, scores)
    e = np.exp(scores - smax[batch])
    denom = np.zeros((nb, H), np.float32)
    np.add.at(denom, batch, e)
    p = e / denom[batch]
    pooled = np.zeros((nb, H, DH), np.float32)
    np.add.at(pooled, batch, p[:, :, None] * v)
    return (pooled.reshape(nb, D) @ W_out.T + b_out).astype(np.float32)


def kernel(**inputs):
    x = np.asarray(inputs["x"], dtype=np.float32)
    batch = np.asarray(inputs["batch"]).astype(np.int64)
    num_graphs = int(np.asarray(inputs["num_graphs"]))
    query = np.asarray(inputs["query"], dtype=np.float32)
    W_in = np.asarray(inputs["W_in"], dtype=np.float32)
    b_in = np.asarray(inputs["b_in"], dtype=np.float32)
    W_out = np.asarray(inputs["W_out"], dtype=np.float32)
    b_out = np.asarray(inputs["b_out"], dtype=np.float32)

    regular = (
        x.shape == (N, D)
        and num_graphs == B
        and batch.shape == (N,)
        and np.array_equal(batch, np.repeat(np.arange(B, dtype=np.int64), NPG))
    )
    if not regular:
        return _numpy_fallback(
            x, batch, num_graphs, query, W_in, b_in, W_out, b_out
        )

    from concourse.bass_utils import run_bass_kernel_spmd

    A4, WvT4, Wout8, bias = _host_prep(query, W_in, b_in, W_out, b_out)

    if "prog" not in _CACHE:
        _CACHE["prog"] = _build(GPC)
    nc = _CACHE["prog"]

    in_maps = _in_maps(x.astype(np.float16), A4, WvT4, Wout8)
    res = run_bass_kernel_spmd(nc, in_maps, list(range(CORES)))
    out = np.concatenate([res.results[c]["out"] for c in range(CORES)], axis=0)
    return (out + bias[None, :]).astype(np.float32)


# revision 34
# speedup vs baseline: 1.2227x; 1.1890x over previous
"""Trainium2 Bass kernel for fused ragged attention pooling (v2).

Problem: single-query multihead attention pooling over segmented nodes.
N=131072 nodes, D=512, B=512 graphs, H=8 heads; graded instance regular:
graph g owns nodes [256*g, 256*(g+1)).

Math (exact, weights folded on host):
    scores[n,h] = x[n,:] @ A[:,h]
    p           = segment softmax(scores)
    S[gh, i]    = sum_{n in g} p[n,h] * x[n,i]
    out         = (S @ WvT per head) @ Wout^T + bias

v2 design (vs v1 which PE-transposed x on chip):
  - Host uploads TWO fp16 copies of x: natural-layout tiles (pooling moving
    operand) and pre-transposed tiles (scores moving operand). This deletes
    the on-chip PE transpose stream AND the PSUM->SBUF staging copies that
    dominated Vector/Scalar time, at the cost of 2x HBM read traffic.
    Both layouts give >=4KB contiguous runs per partition -> near-peak DMA.
  - Softmax: exp on Scalar straight from the scores PSUM (with accum_out
    denominators); normalization is folded into the p-transpose matmul by
    streaming diag(1/den) as the moving operand (regular matmul).
  - Pooling: block-diagonal p16 stationary (16 graphs / 128 cols), exactly
    one x stream through the PE.
"""

import numpy as np

N, D, B, H = 131072, 512, 512, 8
DH = D // H            # 64
CORES = 8
GPC = B // CORES       # graphs per core = 64
NPG = N // B           # nodes per graph = 256
GROUP = 16             # graphs per pooling group
NPGRP = GROUP * NPG    # nodes per group = 4096

_CACHE = {}

CONF = {
    "xtdt": "e3m4",    # dtype of the transposed (scores) copy: f16 | e3m4
}


def _xt_np_dtype(conf):
    if conf["xtdt"] == "e3m4":
        import ml_dtypes

        return ml_dtypes.float8_e3m4
    return np.float16


def _in_maps(x, A4, WvT4, Wout8, conf=None):
    """Per-core input dicts. x must already be fp16 [N, D]."""
    conf = dict(CONF, **(conf or {}))
    x = np.asarray(x, np.float16)
    npc = GPC * NPG
    grps = GPC // GROUP
    ident = np.eye(128, dtype=np.float16)
    xtdt = _xt_np_dtype(conf)
    maps = []
    for c in range(CORES):
        xc = x[c * npc : (c + 1) * npc]
        xg = np.ascontiguousarray(
            xc.reshape(grps, NPGRP // 128, 128, D).transpose(0, 2, 1, 3)
        )
        # pair-major transposed copy: [grp, pair, i_part, c, n_in_pair]
        xt = np.ascontiguousarray(
            xc.reshape(grps, 8, 512, 4, 128).transpose(0, 1, 4, 3, 2)
        ).astype(xtdt)
        maps.append(
            {
                "xg": xg,
                "xt": xt,
                "a4": A4.astype(np.float16),
                "wvt4": WvT4.astype(np.float16),
                "wout8": Wout8,
                "identr": ident,
            }
        )
    return maps


def _build(n_graphs=GPC, repeat=1, variant="full", **overrides):
    """Build + compile the per-core Bass program."""
    conf = dict(CONF, **overrides)
    from contextlib import ExitStack, nullcontext

    import concourse.bacc as bacc
    import concourse.tile as tile
    from concourse import mybir

    F32 = mybir.dt.float32
    F32R = mybir.dt.float32r
    F16 = mybir.dt.float16
    U32 = mybir.dt.uint32
    EXP = mybir.ActivationFunctionType.Exp
    XTD = mybir.dt.float8e3 if conf["xtdt"] == "e3m4" else F16

    assert n_graphs % GROUP == 0
    n_groups = n_graphs // GROUP
    n_pairs = GROUP // 2           # pairs per group = 8
    CHG = 2 * GROUP                # node-chunks per group = 32

    nc = bacc.Bacc("TRN2", target_bir_lowering=False, debug=False)

    xg_d = nc.dram_tensor("xg", [n_groups, 128, CHG, D], F16, kind="ExternalInput")
    xt_d = nc.dram_tensor(
        "xt", [n_groups, n_pairs, 128, 4, 512], XTD, kind="ExternalInput"
    )
    a4_d = nc.dram_tensor("a4", [128, 4, H], F16, kind="ExternalInput")
    wv_d = nc.dram_tensor("wvt4", [128, 4, H, DH], F16, kind="ExternalInput")
    wo_d = nc.dram_tensor("wout8", [DH, H, D], F32R, kind="ExternalInput")
    idr_d = nc.dram_tensor("identr", [128, 128], F16, kind="ExternalInput")
    out_d = nc.dram_tensor("out", [n_graphs, D], F32, kind="ExternalOutput")

    with tile.TileContext(nc) as tc, ExitStack() as ctx:
        const = ctx.enter_context(tc.tile_pool(name="const", bufs=1))
        xt_pool = ctx.enter_context(tc.tile_pool(name="xt", bufs=3))
        xg_pool = ctx.enter_context(tc.tile_pool(name="xgp", bufs=2))
        p16_pool = ctx.enter_context(tc.tile_pool(name="p16", bufs=1))
        small = ctx.enter_context(tc.tile_pool(name="small", bufs=6))
        s2sb_pool = ctx.enter_context(tc.tile_pool(name="s2sb", bufs=2))
        stall_pool = ctx.enter_context(tc.tile_pool(name="stall", bufs=1))
        tail_sb = ctx.enter_context(tc.tile_pool(name="tailsb", bufs=1))
        # PSUM: 8 banks: sc 2 + pnat 2 + s2 2 + tail 1 + pool4 1 = 8
        sc_pool = ctx.enter_context(tc.tile_pool(name="scps", bufs=2, space="PSUM"))
        pnat_pool = ctx.enter_context(tc.tile_pool(name="pnat", bufs=2, space="PSUM"))
        s2_pool = ctx.enter_context(tc.tile_pool(name="s2ps", bufs=2, space="PSUM"))
        tail_ps = ctx.enter_context(tc.tile_pool(name="tailps", bufs=1, space="PSUM"))
        pool4_ps = ctx.enter_context(tc.tile_pool(name="pool4ps", bufs=1, space="PSUM"))

        A4 = const.tile([128, 4, H], F16)
        nc.sync.dma_start(A4[:], a4_d[:])
        WvT4 = const.tile([128, 4, H, DH], F16)
        nc.sync.dma_start(WvT4[:], wv_d[:])
        Wout8 = const.tile([DH, H, D], F32R)
        nc.sync.dma_start(Wout8[:], wo_d[:])
        identr = const.tile([128, 128], F16)
        nc.sync.dma_start(identr[:], idr_d[:])
        ones = const.tile([128, 1], F16)
        nc.vector.memset(ones[:], 1.0)
        nbias = const.tile([128, 1], F32)
        nc.vector.memset(nbias[:], -2.0)

        # persistent block-diagonal p matrices; stripe slots are rewritten
        # every group, the zeros persist from this one-time memset.
        P16 = [
            p16_pool.tile([128, CHG, 128], F16, tag=f"p16_{i}", name=f"p16_{i}")
            for i in range(min(2, n_groups))
        ]
        for t in P16:
            nc.vector.memset(t[:].bitcast(U32), 0)
        # [i, c, h, grp, gl]: pool4's moving slice [:, c, h, :, :] is contiguous
        STall = stall_pool.tile([128, 4, H, n_groups, GROUP], F16)
        POOL4 = pool4_ps.tile([DH, H, n_graphs], F32, name="pool4")

        loop_cm = tc.For_i(0, repeat, 1) if repeat > 1 else nullcontext()
        with loop_cm:
            for grp in range(n_groups):
                # quarter-group loads (2 pairs each) for fine-grained overlap
                xt = xt_pool.tile([128, n_pairs, 4, 512], XTD, tag="xt")
                for q in range(4):
                    nc.sync.dma_start(
                        xt[:, 2 * q : 2 * q + 2, :, :],
                        xt_d[grp][2 * q : 2 * q + 2].rearrange(
                            "a p c d -> p a c d", p=128
                        ),
                    )
                xg = xg_pool.tile([128, CHG, D], F16, tag="xg")
                for q in range(4):
                    nc.scalar.dma_start(
                        xg[:, 8 * q : 8 * q + 8, :], xg_d[grp][:, 8 * q : 8 * q + 8, :]
                    )
                if variant == "dma":
                    continue
                p16 = P16[grp % len(P16)]
                s2ps = s2_pool.tile([128, D], F32, tag="s2")
                # one-pair software-pipeline skew: pair p+1's scores matmuls
                # are emitted BEFORE pair p's transposes/pooling, so the PE
                # has independent work to run while pair p's softmax chain
                # (Scalar exp -> DVE recip/mul) completes.
                pend = {}
                for pstep in range(n_pairs + 1):
                    if pstep < n_pairs:
                        p = pstep
                        scps = sc_pool.tile([H, 2, NPG], F32, tag="sc")
                        for c in range(4):
                            nc.tensor.matmul(
                                scps[:],
                                A4[:, c, :],
                                xt[:, p, c, :],
                                start=(c == 0),
                                stop=(c == 3),
                            )
                        # exp + per-graph denominators, then p = e/den on DVE
                        eT = small.tile([H, 2, NPG], F32, tag="eT")
                        den = small.tile([H, 2, 1], F32, tag="den")
                        rden = small.tile([H, 2, 1], F32, tag="rden")
                        pT = small.tile([H, 2, NPG], F16, tag="pT")
                        # one wide exp (shorter Scalar critical path);
                        # per-graph denominators via DVE reduce instead.
                        nc.scalar.activation(eT[:], scps[:], EXP)
                        from concourse import mybir as _mb
                        for s in range(2):
                            nc.vector.reduce_sum(
                                den[:, s, :], eT[:, s, :],
                                axis=_mb.AxisListType.X,
                            )
                        nc.vector.reciprocal(rden[:], den[:])
                        for s in range(2):
                            nc.vector.tensor_scalar_mul(
                                pT[:, s, :], eT[:, s, :], rden[:, s, :]
                            )
                        pend[p] = pT
                    if pstep >= 1:
                        p = pstep - 1
                        pT = pend.pop(p)
                        # p natural: 4 transpose-path ops [8,128] -> [128,8]
                        pnat = pnat_pool.tile([128, 2, 2, H], F32, tag="pnat")
                        for k in range(4):
                            s, kk = k // 2, k % 2
                            nc.tensor.matmul(
                                pnat[:, s, kk, :],
                                pT[:, s, 128 * kk : 128 * (kk + 1)],
                                identr[0:H, 0:H],
                            )
                        # scatter into block-diag p16: chunk 4p+2s+kk, col h*16+gl
                        from concourse.ap import AP as _AP

                        gl0 = 2 * p
                        dst = _AP(
                            p16.tensor,
                            p16.offset + (4 * p) * 128 + gl0,
                            [list(p16.ap[0])] + [[2 * 128 + 1, 2], [128, 2], [GROUP, H]],
                        )
                        nc.vector.tensor_copy(dst, pnat[:])
                        # pooling: one x stream, block-diag stationary
                        for k in range(4):
                            ch = 4 * p + k
                            nc.tensor.matmul(
                                s2ps[:],
                                p16[:, ch, :],
                                xg[:, ch, :],
                                start=(p == 0 and k == 0),
                                stop=(p == n_pairs - 1 and k == 3),
                            )
                # group tail: evacuate S2, transpose, project (spread per group)
                s2sb = s2sb_pool.tile([128, D], F16, tag="s2sb")
                nc.vector.tensor_copy(s2sb[:], s2ps[:])
                stps = tail_ps.tile([128, 4, H, GROUP], F16, tag="tail")
                for c in range(4):
                    nc.tensor.matmul(
                        stps[:, c, :, :],
                        s2sb[:, 128 * c : 128 * (c + 1)],
                        identr[:],
                        is_transpose=True,
                    )
                nc.scalar.copy(STall[:, :, :, grp, :], stps[:])
                for h in range(H):
                    for c in range(4):
                        nc.tensor.matmul(
                            POOL4[:, h, grp * GROUP : (grp + 1) * GROUP],
                            WvT4[:, c, h, :],
                            STall[:, c, h, grp, :],
                            start=(c == 0),
                            stop=(c == 3),
                        )

            if variant == "dma":
                finz = tail_sb.tile([n_graphs, D], F32, tag="finsb")
                nc.vector.memset(finz[:], 0.0)
                nc.sync.dma_start(out_d[:], finz[:])
            else:
                pool4 = POOL4
                pool4sb = tail_sb.tile([DH, H, n_graphs], F32R, tag="p4sb")
                nc.vector.tensor_copy(pool4sb[:], pool4[:])
                finps = tail_ps.tile([n_graphs, D], F32, tag="tail")
                for h in range(H):
                    nc.tensor.matmul(
                        finps[:],
                        pool4sb[:, h, :],
                        Wout8[:, h, :],
                        start=(h == 0),
                        stop=(h == H - 1),
                    )
                finsb = tail_sb.tile([n_graphs, D], F32, tag="finsb")
                nc.vector.tensor_copy(finsb[:], finps[:])
                nc.sync.dma_start(out_d[:], finsb[:])

    nc.compile()
    _strip_debug(nc)
    return nc


def _strip_debug(nc):
    """Remove source-path debug info so the neuron compile-cache key is
    independent of where this file lives."""
    for fn in nc.m.functions:
        for alloc in fn.allocations:
            try:
                for ml in alloc.memorylocations or []:
                    if getattr(ml, "ant_debug", None) is not None:
                        ml.ant_debug = None
            except Exception:
                pass
        for b in fn.blocks:
            for inst in b.instructions:
                try:
                    if inst.debug is not None:
                        inst.debug = None
                    if inst.bass_addl_debug is not None:
                        inst.bass_addl_debug = None
                except Exception:
                    pass


def _host_prep(query, W_in, b_in, W_out, b_out):
    """Fold the tiny weights into the layouts the device kernel wants."""
    scale = 1.0 / np.sqrt(DH)
    q = ((query @ W_in[:D].T + b_in[:D]) * scale).reshape(H, DH)
    Wk = W_in[D : 2 * D]
    A = (Wk.reshape(H, DH, D) * q[:, :, None]).sum(1).T.astype(np.float32)
    A4 = np.ascontiguousarray(A.reshape(4, 128, H).transpose(1, 0, 2))
    WvT = W_in[2 * D :].T.astype(np.float32)  # [i, j]
    WvT4 = np.ascontiguousarray(WvT.reshape(4, 128, H, DH).transpose(1, 0, 2, 3))
    WoutT = W_out.T.astype(np.float32)  # [j, d]
    Wout8 = np.ascontiguousarray(WoutT.reshape(H, DH, D).transpose(1, 0, 2))
    bias = (W_out @ b_in[2 * D :] + b_out).astype(np.float32)  # [D]
    return A4, WvT4, Wout8, bias


def _numpy_fallback(x, batch, num_graphs, query, W_in, b_in, W_out, b_out):
    """Exact reference math in numpy (handles arbitrary sorted segments)."""
    nb = int(num_graphs)
    scale = 1.0 / np.sqrt(DH)
    q = ((query @ W_in[:D].T + b_in[:D]) * scale).reshape(H, DH)
    k = (x @ W_in[D : 2 * D].T + b_in[D : 2 * D]).reshape(-1, H, DH)
    v = (x @ W_in[2 * D :].T + b_in[2 * D :]).reshape(-1, H, DH)
    scores = np.einsum("nhd,hd->nh", k, q)
    smax = np.full((nb, H), -np.inf, np.float32)
    np.maximum.at(smax, batch, scores)
    e = np.exp(scores - smax[batch])
    denom = np.zeros((nb, H), np.float32)
    np.add.at(denom, batch, e)
    p = e / denom[batch]
    pooled = np.zeros((nb, H, DH), np.float32)
    np.add.at(pooled, batch, p[:, :, None] * v)
    return (pooled.reshape(nb, D) @ W_out.T + b_out).astype(np.float32)


def kernel(**inputs):
    x = np.asarray(inputs["x"], dtype=np.float32)
    batch = np.asarray(inputs["batch"]).astype(np.int64)
    num_graphs = int(np.asarray(inputs["num_graphs"]))
    query = np.asarray(inputs["query"], dtype=np.float32)
    W_in = np.asarray(inputs["W_in"], dtype=np.float32)
    b_in = np.asarray(inputs["b_in"], dtype=np.float32)
    W_out = np.asarray(inputs["W_out"], dtype=np.float32)
    b_out = np.asarray(inputs["b_out"], dtype=np.float32)

    regular = (
        x.shape == (N, D)
        and num_graphs == B
        and batch.shape == (N,)
        and np.array_equal(batch, np.repeat(np.arange(B, dtype=np.int64), NPG))
    )
    if not regular:
        return _numpy_fallback(
            x, batch, num_graphs, query, W_in, b_in, W_out, b_out
        )

    from concourse.bass_utils import run_bass_kernel_spmd

    A4, WvT4, Wout8, bias = _host_prep(query, W_in, b_in, W_out, b_out)

    if "prog" not in _CACHE:
        _CACHE["prog"] = _build(GPC)
    nc = _CACHE["prog"]

    in_maps = _in_maps(x.astype(np.float16), A4, WvT4, Wout8)
    res = run_bass_kernel_spmd(nc, in_maps, list(range(CORES)))
    out = np.concatenate([res.results[c]["out"] for c in range(CORES)], axis=0)
    return (out + bias[None, :]).astype(np.float32)
